# revision 1
# baseline (speedup 1.0000x reference)
"""AttentionV1 Trainium2 Bass kernel.

Data-parallel over batch: 8 images -> 8 NeuronCores. Per core:
  qkv = W_qkv @ x            (1x1 conv, PE, bf16)
  qkv = dwconv3x3(qkv)       (9-tap stencil, scalar_tensor_tensor on DVE)
  qf = q*f, kf = k*f         (DVE)
  G  = qf @ kf^T             (DMA-transpose + PE, accumulated in PSUM)
  sq = rownorms              (ACT Square accum_out)
  attn = softmax(G / (nq nk^T))  (small-tensor phase)
  MT = blockdiag(attn)^T @ W_proj^T   (PE per-head)
  out = MT^T @ v             (PE, fused projection+attn apply)
"""
import sys

for _p in ("/opt/trn_rl_repo",):
    if _p not in sys.path:
        sys.path.insert(0, _p)

import numpy as np

import concourse.bass as bass
import concourse.bacc as bacc
import concourse.mybir as mybir
from concourse.tile import TileContext
from concourse.bass_utils import run_bass_kernel_spmd

F32 = mybir.dt.float32
BF16 = mybir.dt.bfloat16
AL = mybir.AluOpType
AF = mybir.ActivationFunctionType

C = 192          # channels
O = 576          # 3*C
H = 128
W = 128
N = H * W        # 16384
HEADS = 8
CH = 24          # channels per head
TR = 8           # rows per spatial tile
NT = H // TR     # 16 tiles
S = TR * W       # 1024 spatial elems per tile
PW = W + 2       # padded width for stencil
PR = TR + 2      # padded rows (halo)
NCHUNK = S // 128  # 8 transpose chunks per tile

# qkv partition blocks: (abs_start, size). q:128+64, k:128+64, v:96+96
QKV_BLOCKS = [(0, 128), (128, 64), (192, 128), (320, 64), (384, 128), (512, 64)]
# 192-channel partition blocks
CB = [(0, 128), (128, 64)]


def build_nc():
    nc = bacc.Bacc()
    x_d = nc.declare_dram_parameter("x", [C, H, W], F32, isOutput=False)
    f_d = nc.declare_dram_parameter("f", [C, H, W], F32, isOutput=False)
    wq_d = nc.declare_dram_parameter("wq", [C, O], BF16, isOutput=False)     # W_qkv^T
    taps_d = nc.declare_dram_parameter("taps", [O, 9], F32, isOutput=False)
    wp_d = nc.declare_dram_parameter("wp", [C, C], BF16, isOutput=False)     # W_proj^T
    temp_d = nc.declare_dram_parameter("temp", [CH, HEADS], F32, isOutput=False)
    id_d = nc.declare_dram_parameter("ident", [128, 128], F32, isOutput=False)
    out_d = nc.declare_dram_parameter("out", [C, H, W], F32, isOutput=True)
    dbgv_d = nc.declare_dram_parameter("dbg_v", [C, N], F32, isOutput=True)
    dbgg_d = nc.declare_dram_parameter("dbg_g", [C, C], F32, isOutput=True)
    dbgsq_d = nc.declare_dram_parameter("dbg_sq", [C, 2], F32, isOutput=True)
    dbga_d = nc.declare_dram_parameter("dbg_attn", [CH, HEADS * CH], F32, isOutput=True)

    with TileContext(nc) as tc:
        with (
            tc.tile_pool(name="const", bufs=1) as cpool,
            tc.tile_pool(name="vstore", bufs=1) as vpool,
            tc.tile_pool(name="xin", bufs=2) as xpool,
            tc.tile_pool(name="fin", bufs=2) as fpool,
            tc.tile_pool(name="qkv", bufs=2) as qkvpool,
            tc.tile_pool(name="st", bufs=2) as stpool,
            tc.tile_pool(name="tr", bufs=2) as trpool,
            tc.tile_pool(name="fin2", bufs=1) as finpool,
            tc.tile_pool(name="outsb", bufs=3) as outpool,
            tc.tile_pool(name="mm", bufs=2, space="PSUM") as mmpsum,
            tc.tile_pool(name="gram", bufs=1, space="PSUM") as gpsum,
            tc.tile_pool(name="fpsum", bufs=1, space="PSUM") as fpsum,
        ):
            # ---- constants ----
            wq_sb = [cpool.tile([128, O], BF16, tag="wq0", name="wq0"),
                     cpool.tile([64, O], BF16, tag="wq1", name="wq1")]
            nc.sync.dma_start(out=wq_sb[0][:], in_=wq_d[0:128, :])
            nc.sync.dma_start(out=wq_sb[1][:], in_=wq_d[128:192, :])
            taps_sb = []
            for bi, (ms, psz) in enumerate(QKV_BLOCKS):
                t = cpool.tile([psz, 9], F32, tag=f"taps{bi}", name=f"taps{bi}")
                nc.sync.dma_start(out=t[:], in_=taps_d[ms:ms + psz, :])
                taps_sb.append(t)
            # W_proj^T as two 96-row contraction blocks
            wp_sb = [cpool.tile([96, C], BF16, tag="wp0", name="wp0"),
                     cpool.tile([96, C], BF16, tag="wp1", name="wp1")]
            nc.sync.dma_start(out=wp_sb[0][:], in_=wp_d[0:96, :])
            nc.sync.dma_start(out=wp_sb[1][:], in_=wp_d[96:192, :])
            temp_sb = cpool.tile([CH, HEADS], F32, tag="temp", name="temp")
            nc.sync.dma_start(out=temp_sb[:], in_=temp_d[:])

            # persistent v in bf16: two 96-partition blocks
            v_sb = [vpool.tile([128, N], BF16, tag="v0", name="v0"),
                    vpool.tile([64, N], BF16, tag="v1", name="v1")]
            # per-tile sq accum slots
            sq_sb = [cpool.tile([128, NT], F32, tag="sqq0", name="sqq0"),
                     cpool.tile([64, NT], F32, tag="sqq1", name="sqq1"),
                     cpool.tile([128, NT], F32, tag="sqk0", name="sqk0"),
                     cpool.tile([64, NT], F32, tag="sqk1", name="sqk1")]

            # gram psum: accumulated over whole image
            g_ps = [gpsum.tile([128, C], F32, tag="g0", name="g0"),
                    gpsum.tile([64, C], F32, tag="g1", name="g1")]

            total_chunks = NT * NCHUNK

            for t in range(NT):
                r0 = t * TR
                # ---- load x tile with halo rows (bf16 cast via gpsimd dma) ----
                xt = [xpool.tile([128, PR * W], BF16, tag="x0", name="x0"),
                      xpool.tile([64, PR * W], BF16, tag="x1", name="x1")]
                lo = r0 - 1
                hi = r0 + TR + 1  # exclusive
                dlo = max(lo, 0)
                dhi = min(hi, H)
                off = dlo - lo  # 1 if clipped at top else 0
                for ci, (cs, cp) in enumerate(CB):
                    if lo < 0:
                        nc.vector.memset(xt[ci][:, 0:W], 0.0)
                    if hi > H:
                        nc.vector.memset(xt[ci][:, (PR - 1) * W:PR * W], 0.0)
                    nc.gpsimd.dma_start(
                        out=xt[ci][:, off * W:(off + dhi - dlo) * W],
                        in_=x_d[cs:cs + cp, dlo:dhi, :],
                    )
                # ---- load f tile (bf16 cast) ----
                ft = [fpool.tile([128, S], BF16, tag="f0", name="f0"),
                      fpool.tile([64, S], BF16, tag="f1", name="f1")]
                for ci, (cs, cp) in enumerate(CB):
                    nc.gpsimd.dma_start(
                        out=ft[ci][:], in_=f_d[cs:cs + cp, r0:r0 + TR, :])

                # ---- qkv matmul + stencil per block ----
                qk_acc = {}
                for bi, (ms, psz) in enumerate(QKV_BLOCKS):
                    qkv_sb = qkvpool.tile([psz, PR * PW], BF16, tag=f"qkv{bi}", name=f"qkv{bi}")
                    # zero pad columns (x=0 and x=129)
                    q3 = qkv_sb.rearrange("p (r w) -> p r w", w=PW)
                    nc.vector.memset(q3[:, :, 0:1], 0.0)
                    nc.vector.memset(q3[:, :, PW - 1:PW], 0.0)
                    # matmul in row-chunks of <=4 rows (psum 512 limit)
                    for c0, csz in ((0, 4), (4, 4), (8, 2)):
                        ps = mmpsum.tile([psz, 4 * W], F32, tag="mmps", name="mmps")
                        nc.tensor.matmul(
                            ps[:, :csz * W],
                            wq_sb[0][:, ms:ms + psz],
                            xt[0][:, c0 * W:(c0 + csz) * W],
                            start=True, stop=False)
                        nc.tensor.matmul(
                            ps[:, :csz * W],
                            wq_sb[1][:, ms:ms + psz],
                            xt[1][:, c0 * W:(c0 + csz) * W],
                            start=False, stop=True)
                        # copy into padded sbuf (cast to bf16)
                        nc.vector.tensor_copy(
                            q3[:, c0:c0 + csz, 1:1 + W],
                            ps[:, :csz * W].rearrange("p (r w) -> p r w", w=W))
                    # ---- 9-tap stencil ----
                    if bi < 4:
                        acc = stpool.tile([psz, S], BF16, tag=f"acc{bi}", name=f"acc{bi}")
                        acc_ap = acc.rearrange("p (r w) -> p r w", w=W)
                    else:
                        acc_ap = v_sb[bi - 4][:, t * S:(t + 1) * S].rearrange(
                            "p (r w) -> p r w", w=W)
                    ti = 0
                    for dy in (-1, 0, 1):
                        for dx in (-1, 0, 1):
                            src = q3[:, 1 + dy:1 + dy + TR, 1 + dx:1 + dx + W]
                            w_ap = taps_sb[bi][:, ti:ti + 1]
                            if ti == 0:
                                nc.vector.tensor_scalar_mul(acc_ap, src, w_ap)
                            else:
                                nc.vector.scalar_tensor_tensor(
                                    acc_ap, src, w_ap, acc_ap,
                                    op0=AL.mult, op1=AL.add)
                            ti += 1
                    if bi < 4:
                        qk_acc[bi] = acc

                # ---- qf/kf multiply + norms + transpose ----
                qf = [stpool.tile([128, S], BF16, tag="qf0", name="qf0"),
                      stpool.tile([64, S], BF16, tag="qf1", name="qf1"),
                      stpool.tile([128, S], BF16, tag="kf0", name="kf0"),
                      stpool.tile([64, S], BF16, tag="kf1", name="kf1")]
                scr = [stpool.tile([128, S], BF16, tag="scr0", name="scr0"),
                       stpool.tile([64, S], BF16, tag="scr1", name="scr1")]
                for i in range(4):
                    nc.vector.tensor_mul(qf[i][:], qk_acc[i][:], ft[i % 2][:])
                    nc.scalar.activation(
                        scr[i % 2][:], qf[i][:], AF.Square,
                        accum_out=sq_sb[i][:, t:t + 1])

                qfT = trpool.tile([128, NCHUNK * C], BF16, tag="qfT", name="qfT")
                kfT = trpool.tile([128, NCHUNK * C], BF16, tag="kfT", name="kfT")
                for j in range(NCHUNK):
                    for i, dst in ((0, qfT), (1, qfT), (2, kfT), (3, kfT)):
                        cs, cp = CB[i % 2]
                        nc.sync.dma_start_transpose(
                            dst[:, j * C + cs: j * C + cs + cp],
                            qf[i][:, j * 128:(j + 1) * 128])
                    # ---- gram accumulation ----
                    g = t * NCHUNK + j
                    nc.tensor.matmul(
                        g_ps[0][:],
                        qfT[:, j * C: j * C + 128],
                        kfT[:, j * C: (j + 1) * C],
                        start=(g == 0), stop=(g == total_chunks - 1))
                    nc.tensor.matmul(
                        g_ps[1][:],
                        qfT[:, j * C + 128: (j + 1) * C],
                        kfT[:, j * C: (j + 1) * C],
                        start=(g == 0), stop=(g == total_chunks - 1))

            # ================= final small-tensor phase =================
            # norms: rsq = 1/sqrt(max(sum sq, eps))
            rq = []
            for i in range(4):
                psz = [128, 64, 128, 64][i]
                sq1 = finpool.tile([psz, 1], F32, tag=f"sq1_{i}", name=f"sq1_{i}")
                nc.vector.tensor_reduce(
                    sq1[:], sq_sb[i][:], axis=mybir.AxisListType.X, op=AL.add)
                nc.vector.tensor_scalar_max(sq1[:], sq1[:], 1e-24)
                nq = finpool.tile([psz, 1], F32, tag=f"nq_{i}", name=f"nq_{i}")
                nc.scalar.activation(nq[:], sq1[:], AF.Sqrt)
                r = finpool.tile([psz, 1], F32, tag=f"rq_{i}", name=f"rq_{i}")
                nc.vector.reciprocal(r[:], nq[:])
                rq.append(r)

            # G with rq applied (per-partition scale), fp32 in sbuf
            G_sb = [finpool.tile([128, C], F32, tag="G0", name="G0"),
                    finpool.tile([64, C], F32, tag="G1", name="G1")]
            nc.vector.tensor_scalar_mul(G_sb[0][:], g_ps[0][:], rq[0][:])
            nc.vector.tensor_scalar_mul(G_sb[1][:], g_ps[1][:], rq[1][:])

            # PE transpose G -> GT in psum
            ident = cpool.tile([128, 128], F32, tag="ident", name="ident")
            nc.sync.dma_start(out=ident[:], in_=id_d[:])
            gt_ps = [fpsum.tile([128, C], F32, tag="gt0", name="gt0"),
                     fpsum.tile([64, C], F32, tag="gt1", name="gt1")]
            nc.tensor.matmul(gt_ps[0][:, 0:128], G_sb[0][:, 0:128], ident[:],
                             is_transpose=True, start=True, stop=True)
            nc.tensor.matmul(gt_ps[0][:, 128:192], G_sb[1][:, 0:128],
                             ident[0:64, 0:64], is_transpose=True,
                             start=True, stop=True)
            nc.tensor.matmul(gt_ps[1][:, 0:128], G_sb[0][:, 128:192], ident[:],
                             is_transpose=True, start=True, stop=True)
            nc.tensor.matmul(gt_ps[1][:, 128:192], G_sb[1][:, 128:192],
                             ident[0:64, 0:64], is_transpose=True,
                             start=True, stop=True)

            # rk packed per head (with temperature folded in)
            rkp = finpool.tile([32, HEADS], F32, tag="rkp", name="rkp")
            nc.vector.memset(rkp[:], 0.0)
            for h in range(HEADS):
                a0 = h * CH          # abs row in [0,192)
                a1 = a0 + CH
                if a1 <= 128:
                    nc.sync.dma_start(out=rkp[0:CH, h:h + 1],
                                      in_=rq[2][a0:a1, :])
                elif a0 >= 128:
                    nc.sync.dma_start(out=rkp[0:CH, h:h + 1],
                                      in_=rq[3][a0 - 128:a1 - 128, :])
                else:
                    m = 128 - a0
                    nc.sync.dma_start(out=rkp[0:m, h:h + 1],
                                      in_=rq[2][a0:128, :])
                    nc.sync.dma_start(out=rkp[m:CH, h:h + 1],
                                      in_=rq[3][0:a1 - 128, :])
            nc.vector.tensor_mul(rkp[0:CH, :], rkp[0:CH, :], temp_sb[:])

            # stage GT into SBUF (DMA cannot read PSUM)
            gt_sb = [finpool.tile([128, C], F32, tag="gts0", name="gts0"),
                     finpool.tile([64, C], F32, tag="gts1", name="gts1")]
            nc.vector.tensor_copy(gt_sb[0][:], gt_ps[0][:])
            nc.vector.tensor_copy(gt_sb[1][:], gt_ps[1][:])
            # assemble A_T (32, HEADS*32): A_T[d, h*32+c] = GT[24h+d, 24h+c] * rk
            at = finpool.tile([32, HEADS * 32], F32, tag="at", name="at")
            nc.vector.memset(at[:], 0.0)
            for h in range(HEADS):
                a0 = h * CH
                a1 = a0 + CH
                col = slice(a0, a1)
                if a1 <= 128:
                    nc.sync.dma_start(out=at[0:CH, h * 32:h * 32 + CH],
                                      in_=gt_sb[0][a0:a1, col])
                elif a0 >= 128:
                    nc.sync.dma_start(out=at[0:CH, h * 32:h * 32 + CH],
                                      in_=gt_sb[1][a0 - 128:a1 - 128, col])
                else:
                    m = 128 - a0
                    nc.sync.dma_start(out=at[0:m, h * 32:h * 32 + CH],
                                      in_=gt_sb[0][a0:128, col])
                    nc.sync.dma_start(out=at[m:CH, h * 32:h * 32 + CH],
                                      in_=gt_sb[1][0:a1 - 128, col])
                nc.vector.tensor_scalar_mul(
                    at[0:CH, h * 32:h * 32 + CH],
                    at[0:CH, h * 32:h * 32 + CH],
                    rkp[0:CH, h:h + 1])

            # transpose per-head 32x32 blocks back: A[c, h*32+d]
            a_sb = finpool.tile([32, HEADS * 32], F32, tag="a", name="a")
            nc.vector.transpose(a_sb[:], at[:])
            # exp
            e_sb = finpool.tile([32, HEADS * 32], F32, tag="e", name="e")
            nc.scalar.activation(e_sb[:], a_sb[:], AF.Exp)
            # per-head row sums over d (exclude junk cols >= CH)
            e3 = e_sb.rearrange("p (h d) -> p h d", d=32)
            sums = finpool.tile([CH, HEADS], F32, tag="sums", name="sums")
            nc.vector.tensor_reduce(
                sums[:], e3[0:CH, :, 0:CH], axis=mybir.AxisListType.X, op=AL.add)
            rs = finpool.tile([CH, HEADS], F32, tag="rs", name="rs")
            nc.vector.reciprocal(rs[:], sums[:])
            # attn (bf16) packed (24, h*24)
            attn = finpool.tile([CH, HEADS * CH], BF16, tag="attn", name="attn")
            for h in range(HEADS):
                nc.vector.tensor_scalar_mul(
                    attn[:, h * CH:(h + 1) * CH],
                    e_sb[0:CH, h * 32:h * 32 + CH],
                    rs[:, h:h + 1])

            # ---- debug dumps ----
            for ci, (cs, cp) in enumerate(CB):
                for jj in range(N // 512):
                    dv2 = outpool.tile([cp, 512], F32, tag="osb0", name="dbgv2")
                    nc.vector.tensor_copy(dv2[:], v_sb[ci][:, jj * 512:(jj + 1) * 512])
                    nc.sync.dma_start(out=dbgv_d[cs:cs + cp, jj * 512:(jj + 1) * 512],
                                      in_=dv2[:])
            dbg_g_sb = [finpool.tile([128, C], F32, tag="dbgg0", name="dbgg0"),
                        finpool.tile([64, C], F32, tag="dbgg1", name="dbgg1")]
            nc.vector.tensor_copy(dbg_g_sb[0][:], g_ps[0][:])
            nc.vector.tensor_copy(dbg_g_sb[1][:], g_ps[1][:])
            nc.sync.dma_start(out=dbgg_d[0:128, :], in_=dbg_g_sb[0][:])
            nc.sync.dma_start(out=dbgg_d[128:192, :], in_=dbg_g_sb[1][:])
            nc.sync.dma_start(out=dbgsq_d[0:128, 0:1], in_=rq[0][:])
            nc.sync.dma_start(out=dbgsq_d[128:192, 0:1], in_=rq[1][:])
            nc.sync.dma_start(out=dbgsq_d[0:128, 1:2], in_=rq[2][:])
            nc.sync.dma_start(out=dbgsq_d[128:192, 1:2], in_=rq[3][:])
            dbg_at = finpool.tile([CH, HEADS * CH], F32, tag="dbgat", name="dbgat")
            nc.vector.tensor_copy(dbg_at[:], attn[:])
            nc.sync.dma_start(out=dbga_d[:], in_=dbg_at[:])
            # dense blockdiag(attn) as two 96-row contraction blocks
            bd = [finpool.tile([96, C], BF16, tag="bd0", name="bd0"),
                  finpool.tile([96, C], BF16, tag="bd1", name="bd1")]
            nc.vector.memset(bd[0][:], 0.0)
            nc.vector.memset(bd[1][:], 0.0)
            for h in range(HEADS):
                nc.sync.dma_start(
                    out=bd[h // 4][(h % 4) * CH:(h % 4) * CH + CH,
                                   h * CH:(h + 1) * CH],
                    in_=attn[:, h * CH:(h + 1) * CH])
            # MT = blockdiag(attn)^T @ W_proj^T  (rows = v channels, 128+64)
            mt_ps = [fpsum.tile([128, C], F32, tag="mt0", name="mt0"),
                     fpsum.tile([64, C], F32, tag="mt1", name="mt1")]
            for mi, msl in enumerate((slice(0, 128), slice(128, 192))):
                for k in range(2):
                    nc.tensor.matmul(mt_ps[mi][:], bd[k][:, msl], wp_sb[k][:],
                                     start=(k == 0), stop=(k == 1))
            mt_sb = [finpool.tile([128, C], BF16, tag="mt_sb0", name="mt_sb0"),
                     finpool.tile([64, C], BF16, tag="mt_sb1", name="mt_sb1")]
            nc.vector.tensor_copy(mt_sb[0][:], mt_ps[0][:])
            nc.vector.tensor_copy(mt_sb[1][:], mt_ps[1][:])

            # ---- output: out = MT^T @ v (fused attn-apply + projection) ----
            for j in range(N // 512):
                col = slice(j * 512, (j + 1) * 512)
                for mi, (msz, msl) in enumerate(((128, slice(0, 128)),
                                                 (64, slice(128, 192)))):
                    ps = mmpsum.tile([msz, 512], F32, tag="mmps", name="mmps")
                    nc.tensor.matmul(ps[:], mt_sb[0][:, msl], v_sb[0][:, col],
                                     start=True, stop=False)
                    nc.tensor.matmul(ps[:], mt_sb[1][:, msl], v_sb[1][:, col],
                                     start=False, stop=True)
                    osb = outpool.tile([msz, 512], F32, tag=f"osb{mi}", name=f"osb{mi}")
                    nc.vector.tensor_copy(osb[:], ps[:])
                    cs = 0 if mi == 0 else 128
                    nc.sync.dma_start(
                        out=out_d.rearrange("c h w -> c (h w)")[cs:cs + msz, col],
                        in_=osb[:])
    nc.finalize()
    return nc


_NC_CACHE = {}


def kernel(x, feature, W_qkv, W_dw, W_proj, temperature):
    import ml_dtypes
    b = x.shape[0]
    wq = np.ascontiguousarray(np.asarray(W_qkv, np.float32).T).astype(
        ml_dtypes.bfloat16)
    taps = np.ascontiguousarray(
        np.asarray(W_dw, np.float32).reshape(O, 9))
    wp = np.ascontiguousarray(np.asarray(W_proj, np.float32).T).astype(
        ml_dtypes.bfloat16)
    temp = np.broadcast_to(
        np.asarray(temperature, np.float32).reshape(1, HEADS), (CH, HEADS))
    temp = np.ascontiguousarray(temp)

    if "nc" not in _NC_CACHE:
        _NC_CACHE["nc"] = build_nc()
    nc = _NC_CACHE["nc"]

    in_maps = []
    for i in range(b):
        in_maps.append({
            "x": np.ascontiguousarray(np.asarray(x[i], np.float32)),
            "f": np.ascontiguousarray(np.asarray(feature[i], np.float32)),
            "wq": wq, "taps": taps, "wp": wp, "temp": temp,
            "ident": np.eye(128, dtype=np.float32),
        })
    res = run_bass_kernel_spmd(nc, in_maps, list(range(b)))
    outs = [np.asarray(r["out"], np.float32).reshape(C, H, W)
            for r in res.results]
    return np.stack(outs, axis=0)



# revision 8
# speedup vs baseline: 2.3635x; 2.3635x over previous
"""AttentionV1 Trainium2 Bass kernel (v2).

Data-parallel over batch: 8 images -> 8 NeuronCores. Per core:
  qkv = W_qkv @ x            (1x1 conv, PE, bf16, permuted 5-block layout)
  qkv = dwconv3x3(qkv)       (q,k: 9-tap STT on DVE at 2x; v: diag-matmul on PE)
  qf = q*f, kf = k*f         (DVE)
  G  = qf @ kf^T             (PE transpose-via-identity + PE gram)
  attn = softmax(G / (nq nk^T))  (small-tensor phase)
  out = (blockdiag(attn)^T @ W_proj^T)^T @ v   (PE)

Channel blocks (output-channel permutation of W_qkv/taps):
  B0 = q[0:128], B1 = q[128:192] || k[128:192], B2 = k[0:128],
  B3 = v[0:128], B4 = v[128:192]
"""
import sys

for _p in ("/opt/trn_rl_repo",):
    if _p not in sys.path:
        sys.path.insert(0, _p)

import numpy as np

import concourse.bass as bass
import concourse.bacc as bacc
import concourse.mybir as mybir
from concourse.tile import TileContext
from concourse.bass_utils import run_bass_kernel_spmd

F32 = mybir.dt.float32
BF16 = mybir.dt.bfloat16
AL = mybir.AluOpType
AF = mybir.ActivationFunctionType

C = 192          # channels
O = 576          # 3*C
H = 128
W = 128
N = H * W        # 16384
HEADS = 8
CH = 24          # channels per head
TR = 8           # rows per spatial tile
NT = H // TR     # 16 tiles
S = TR * W       # 1024 spatial elems per tile
PR = TR + 2      # padded rows (halo)
PW = W + 4       # padded width: cols [2,130) hold x in [0,128)
NCHUNK = S // 128  # 8 transpose chunks per tile

# channel blocks: (tag, psz). B0=q_lo B1=q_hi||k_hi B2=k_lo B3=v_lo B4=v_hi
BLK = [128, 128, 128, 128, 64]
QK_BLOCKS = (0, 1, 2)
V_BLOCKS = (3, 4)
TAP_OFF = [(3 * (dy + 1) + (dx + 1), dy, dx)
           for dy in (-1, 0, 1) for dx in (-1, 0, 1)]


def build_nc():
    nc = bacc.Bacc()
    x_d = nc.declare_dram_parameter("x", [C, H, W], F32, isOutput=False)
    f_d = nc.declare_dram_parameter("f", [C, H, W], F32, isOutput=False)
    wq_d = nc.declare_dram_parameter("wq", [C, O], BF16, isOutput=False)   # W_qkv^T perm
    taps_d = nc.declare_dram_parameter("taps", [O, 9], F32, isOutput=False)  # perm
    vd3_d = nc.declare_dram_parameter("vdiag3", [128, 9 * 128], BF16, isOutput=False)
    vd4_d = nc.declare_dram_parameter("vdiag4", [64, 9 * 64], BF16, isOutput=False)
    wp_d = nc.declare_dram_parameter("wp", [C, C], BF16, isOutput=False)   # W_proj^T
    temp_d = nc.declare_dram_parameter("temp", [CH, HEADS], F32, isOutput=False)
    idb_d = nc.declare_dram_parameter("identb", [128, 128], BF16, isOutput=False)
    idf_d = nc.declare_dram_parameter("identf", [128, 128], F32, isOutput=False)
    out_d = nc.declare_dram_parameter("out", [C, N], BF16, isOutput=True)

    with TileContext(nc) as tc:
        with (
            tc.tile_pool(name="const", bufs=1) as cpool,
            tc.tile_pool(name="vstore", bufs=1) as vpool,
            tc.tile_pool(name="xin", bufs=2) as xpool,
            tc.tile_pool(name="fin", bufs=2) as fpool,
            tc.tile_pool(name="qkv", bufs=2) as qkvpool,
            tc.tile_pool(name="st", bufs=2) as stpool,
            tc.tile_pool(name="scr", bufs=2) as scrpool,
            tc.tile_pool(name="tsb", bufs=3) as tsbpool,
            tc.tile_pool(name="fin2", bufs=1) as finpool,
            tc.tile_pool(name="outsb", bufs=3) as outpool,
            tc.tile_pool(name="mm", bufs=2, space="PSUM") as mmpsum,
            tc.tile_pool(name="vps", bufs=1, space="PSUM") as vpsum,
            tc.tile_pool(name="tps", bufs=1, space="PSUM") as tpsum,
            tc.tile_pool(name="gram", bufs=1, space="PSUM") as gpsum,
        ):
            # ---- constants ----
            wq_sb = [cpool.tile([128, O], BF16, tag="wq0", name="wq0"),
                     cpool.tile([64, O], BF16, tag="wq1", name="wq1")]
            nc.sync.dma_start(out=wq_sb[0][:], in_=wq_d[0:128, :])
            nc.sync.dma_start(out=wq_sb[1][:], in_=wq_d[128:192, :])
            taps_sb = []
            ms = 0
            for bi, psz in enumerate(BLK):
                t = cpool.tile([psz, 9], F32, tag=f"taps{bi}", name=f"taps{bi}")
                nc.sync.dma_start(out=t[:], in_=taps_d[ms:ms + psz, :])
                taps_sb.append(t)
                ms += psz
            vd3 = cpool.tile([128, 9 * 128], BF16, tag="vd3", name="vd3")
            nc.sync.dma_start(out=vd3[:], in_=vd3_d[:])
            vd4 = cpool.tile([64, 9 * 64], BF16, tag="vd4", name="vd4")
            nc.sync.dma_start(out=vd4[:], in_=vd4_d[:])
            wp_sb = [cpool.tile([96, C], BF16, tag="wp0", name="wp0"),
                     cpool.tile([96, C], BF16, tag="wp1", name="wp1")]
            nc.sync.dma_start(out=wp_sb[0][:], in_=wp_d[0:96, :])
            nc.sync.dma_start(out=wp_sb[1][:], in_=wp_d[96:192, :])
            temp_sb = cpool.tile([CH, HEADS], F32, tag="temp", name="temp")
            nc.sync.dma_start(out=temp_sb[:], in_=temp_d[:])
            identb = cpool.tile([128, 128], BF16, tag="identb", name="identb")
            nc.sync.dma_start(out=identb[:], in_=idb_d[:])
            identf = cpool.tile([128, 128], F32, tag="identf", name="identf")
            nc.sync.dma_start(out=identf[:], in_=idf_d[:])

            # persistent v (bf16): B3 128ch + B4 64ch
            v_sb = [vpool.tile([128, N], BF16, tag="v0", name="v0"),
                    vpool.tile([64, N], BF16, tag="v1", name="v1")]
            # per-tile square accum (q,k blocks only)
            sq_sb = [cpool.tile([128, NT], F32, tag=f"sq{i}", name=f"sq{i}")
                     for i in range(3)]
            # gram accumulators packed into one PSUM bank:
            # g0 = [128, 0:192], g1 = [0:64, 256:448]
            g_all = gpsum.tile([128, 512], F32, tag="g", name="g")
            g_ps = [g_all[:, 0:C], g_all[0:64, 256:256 + C]]

            for t in range(NT):
                r0 = t * TR
                # ---- load x tile with halo rows (bf16 cast via gpsimd dma) ----
                xt = [xpool.tile([128, PR * W], BF16, tag="x0", name="x0"),
                      xpool.tile([64, PR * W], BF16, tag="x1", name="x1")]
                lo = r0 - 1
                hi = r0 + TR + 1  # exclusive
                dlo = max(lo, 0)
                dhi = min(hi, H)
                off = dlo - lo
                for ci, (cs, cp) in enumerate(((0, 128), (128, 64))):
                    if lo < 0:
                        nc.vector.memset(xt[ci][:, 0:W], 0.0)
                    if hi > H:
                        nc.vector.memset(xt[ci][:, (PR - 1) * W:PR * W], 0.0)
                    nc.gpsimd.dma_start(
                        out=xt[ci][:, off * W:(off + dhi - dlo) * W],
                        in_=x_d[cs:cs + cp, dlo:dhi, :],
                    )
                # ---- load f tiles ----
                # ft_a = f[0:128] (for B0 q_lo and B2 k_lo)
                # ft_b = f[128:192] twice (for B1 = q_hi || k_hi)
                ft_a = fpool.tile([128, S], BF16, tag="fa", name="fa")
                nc.gpsimd.dma_start(out=ft_a[:], in_=f_d[0:128, r0:r0 + TR, :])
                ft_b = fpool.tile([128, S], BF16, tag="fb", name="fb")
                nc.gpsimd.dma_start(out=ft_b[0:64, :], in_=f_d[128:192, r0:r0 + TR, :])
                nc.gpsimd.dma_start(out=ft_b[64:128, :], in_=f_d[128:192, r0:r0 + TR, :])

                # ---- qkv matmul (5 blocks, 10 halo rows each) + psum->sbuf ----
                sb = []
                sb2 = []
                ms = 0
                for bi, psz in enumerate(BLK):
                    q_sb = qkvpool.tile([psz, PR * PW], BF16, tag=f"sb{bi}",
                                        name=f"sb{bi}")
                    q3 = q_sb.rearrange("p (r w) -> p r w", w=PW)
                    sb.append(q3)
                    if bi in QK_BLOCKS:
                        q_sb2 = qkvpool.tile([psz, PR * PW], BF16, tag=f"sc{bi}",
                                             name=f"sc{bi}")
                        q32 = q_sb2.rearrange("p (r w) -> p r w", w=PW)
                        sb2.append(q32)
                        # zero pads read by shifted taps: cols 0 and 129
                        nc.vector.memset(q32[:, :, 0:1], 0.0)
                        nc.vector.memset(q32[:, :, 129:130], 0.0)
                    else:
                        sb2.append(None)
                        # v blocks: PE reads cols [1,131); zero cols 1 and 130
                        nc.vector.memset(q3[:, :, 1:2], 0.0)
                        nc.vector.memset(q3[:, :, 130:131], 0.0)
                    # matmul in row-chunks (psum bank = 512 f32)
                    for c0, csz in ((0, 4), (4, 4), (8, 2)):
                        ps = mmpsum.tile([psz, 4 * W], F32, tag="mmps", name="mmps")
                        nc.tensor.matmul(
                            ps[:, :csz * W],
                            wq_sb[0][:, ms:ms + psz],
                            xt[0][:, c0 * W:(c0 + csz) * W],
                            start=True, stop=False)
                        nc.tensor.matmul(
                            ps[:, :csz * W],
                            wq_sb[1][:, ms:ms + psz],
                            xt[1][:, c0 * W:(c0 + csz) * W],
                            start=False, stop=True)
                        ps3 = ps[:, :csz * W].rearrange("p (r w) -> p r w", w=W)
                        # aligned copy: x lands at cols [2,130)
                        nc.scalar.activation(
                            q3[:, c0:c0 + csz, 2:2 + W], ps3, AF.Copy)
                        if bi in QK_BLOCKS:
                            # shifted copy: x lands at cols [1,129)
                            nc.scalar.activation(
                                q32[:, c0:c0 + csz, 1:1 + W], ps3, AF.Copy)
                    ms += psz

                # ---- q,k stencil on DVE (all taps 2x via alignment) ----
                st = []
                for bi in QK_BLOCKS:
                    psz = BLK[bi]
                    acc = stpool.tile([psz, S], BF16, tag=f"st{bi}", name=f"st{bi}")
                    acc_ap = acc.rearrange("p (r w) -> p r w", w=W)
                    first = True
                    for ti, dy, dx in TAP_OFF:
                        if dx == 0:
                            src = sb[bi][:, 1 + dy:1 + dy + TR, 2:2 + W]
                        else:
                            # sb2[c] = sb[c+1]; value(x+dx) = sb2 col 1+x+dx
                            src = sb2[bi][:, 1 + dy:1 + dy + TR,
                                          1 + dx:1 + dx + W]
                        w_ap = taps_sb[bi][:, ti:ti + 1]
                        if first:
                            nc.vector.tensor_scalar_mul(acc_ap, src, w_ap)
                            first = False
                        else:
                            nc.vector.scalar_tensor_tensor(
                                acc_ap, src, w_ap, acc_ap,
                                op0=AL.mult, op1=AL.add)
                    st.append(acc)

                # ---- qf/kf multiply (in place) + squares ----
                fts = [ft_a, ft_b, ft_a]
                for i, bi in enumerate(QK_BLOCKS):
                    nc.vector.tensor_mul(st[i][:], st[i][:], fts[i][:])
                    scr = scrpool.tile([128, S], BF16, tag=f"scr{i}",
                                       name=f"scr{i}")
                    nc.scalar.activation(
                        scr[:], st[i][:], AF.Square,
                        accum_out=sq_sb[i][:, t:t + 1])

                # ---- transposes (PE identity-matmul) + gram accumulation,
                # with v-stencil diag-matmuls interleaved as PE filler so the
                # bufs=1 qt/kt psum and the ACT casts pipeline cleanly ----
                # v-stencil batches: 36 MMs (2 halves x 9 taps x 2 blocks)
                # split across the 8 chunks: chunk j runs taps for
                # half = j//4, tap-pairs within.
                vps = {}
                for j in range(NCHUNK):
                    g = t * NCHUNK + j
                    col = slice(j * 128, (j + 1) * 128)
                    qt_ps = tpsum.tile([128, C], F32, tag="qt", name="qt")
                    kt_ps = tpsum.tile([128, C], F32, tag="kt", name="kt")
                    # qT: cols 0:128 = q_lo (B0), cols 128:192 = q_hi (B1 lo)
                    nc.tensor.matmul(qt_ps[:, 0:128], st[0][:, col],
                                     identb[:], start=True, stop=True)
                    nc.tensor.matmul(qt_ps[:, 128:192], st[1][0:64, col],
                                     identb[0:64, 0:64], start=True, stop=True)
                    # kT: cols 0:128 = k_lo (B2), cols 128:192 = k_hi (B1 hi)
                    nc.tensor.matmul(kt_ps[:, 0:128], st[2][:, col],
                                     identb[:], start=True, stop=True)
                    nc.tensor.matmul(kt_ps[:, 128:192], st[1][64:128, col],
                                     identb[64:128, 64:128], start=True, stop=True)
                    qt_sb = tsbpool.tile([128, C], BF16, tag="qts", name="qts")
                    kt_sb = tsbpool.tile([128, C], BF16, tag="kts", name="kts")
                    nc.scalar.activation(qt_sb[:], qt_ps[:], AF.Copy)
                    nc.scalar.activation(kt_sb[:], kt_ps[:], AF.Copy)
                    # --- v-stencil filler: ~2-3 taps of (B3+B4) per chunk ---
                    half = j // 4
                    jj = j % 4
                    if jj == 0:
                        vps["vp3"] = vpsum.tile([128, 512], F32, tag="vp3",
                                                name="vp3")
                        vps["vp4"] = vpsum.tile([64, 512], F32, tag="vp4",
                                                name="vp4")
                    tap_sl = ((0, 2), (2, 4), (4, 6), (6, 9))[jj]
                    for ti in range(tap_sl[0], tap_sl[1]):
                        _, dy, dx = TAP_OFF[ti]
                        r_lo = 1 + dy + 4 * half
                        rhs3 = sb[3][:, r_lo:r_lo + 4, 2 + dx:2 + dx + W]
                        nc.tensor.matmul(
                            vps["vp3"][:], vd3[:, ti * 128:(ti + 1) * 128],
                            rhs3, start=(ti == 0), stop=(ti == 8))
                        rhs4 = sb[4][:, r_lo:r_lo + 4, 2 + dx:2 + dx + W]
                        nc.tensor.matmul(
                            vps["vp4"][:], vd4[:, ti * 64:(ti + 1) * 64],
                            rhs4, start=(ti == 0), stop=(ti == 8))
                    if jj == 3:
                        cdst = slice(t * S + half * 512,
                                     t * S + (half + 1) * 512)
                        nc.scalar.activation(v_sb[0][:, cdst], vps["vp3"][:],
                                             AF.Copy)
                        nc.scalar.activation(v_sb[1][:, cdst], vps["vp4"][:],
                                             AF.Copy)
                    # --- gram accumulation for this chunk ---
                    nc.tensor.matmul(
                        g_ps[0][:], qt_sb[:, 0:128], kt_sb[:],
                        start=(g == 0), stop=(g == NT * NCHUNK - 1))
                    nc.tensor.matmul(
                        g_ps[1][:], qt_sb[:, 128:192], kt_sb[:],
                        start=(g == 0), stop=(g == NT * NCHUNK - 1))

            # ================= final small-tensor phase =================
            # per-block norms: r = 1/sqrt(max(sum sq, eps))
            rb = []
            for i in range(3):
                sq1 = finpool.tile([128, 1], F32, tag=f"sq1_{i}", name=f"sq1_{i}")
                nc.vector.tensor_reduce(
                    sq1[:], sq_sb[i][:], axis=mybir.AxisListType.X, op=AL.add)
                nc.vector.tensor_scalar_max(sq1[:], sq1[:], 1e-24)
                nq = finpool.tile([128, 1], F32, tag=f"nq_{i}", name=f"nq_{i}")
                nc.scalar.activation(nq[:], sq1[:], AF.Sqrt)
                r = finpool.tile([128, 1], F32, tag=f"rq_{i}", name=f"rq_{i}")
                nc.vector.reciprocal(r[:], nq[:])
                rb.append(r)
            # rq rows: q0:128 = rb[0], q128:192 = rb[1][0:64]
            # rk rows: k0:128 = rb[2], k128:192 = rb[1][64:128]

            # G with rq applied (per-partition row scale)
            G_sb = [finpool.tile([128, C], F32, tag="G0", name="G0"),
                    finpool.tile([64, C], F32, tag="G1", name="G1")]
            nc.vector.tensor_scalar_mul(G_sb[0][:], g_ps[0][:], rb[0][:])
            nc.vector.tensor_scalar_mul(G_sb[1][:], g_ps[1][:], rb[1][0:64, :])

            # PE transpose G -> GT in psum (k rows, q cols); reuse v psum tags
            gt0_t = vpsum.tile([128, 512], F32, tag="vp3", name="gt0")
            gt1_t = vpsum.tile([64, 512], F32, tag="vp4", name="gt1")
            gt_ps = [gt0_t[:, 0:C], gt1_t[:, 0:C]]
            nc.tensor.matmul(gt_ps[0][:, 0:128], G_sb[0][:, 0:128], identf[:],
                             is_transpose=True, start=True, stop=True)
            nc.tensor.matmul(gt_ps[0][:, 128:192], G_sb[1][:, 0:128],
                             identf[0:64, 0:64], is_transpose=True,
                             start=True, stop=True)
            nc.tensor.matmul(gt_ps[1][:, 0:128], G_sb[0][:, 128:192], identf[:],
                             is_transpose=True, start=True, stop=True)
            nc.tensor.matmul(gt_ps[1][:, 128:192], G_sb[1][:, 128:192],
                             identf[0:64, 0:64], is_transpose=True,
                             start=True, stop=True)

            # rk packed per head (with temperature folded in)
            rkp = finpool.tile([32, HEADS], F32, tag="rkp", name="rkp")
            nc.vector.memset(rkp[:], 0.0)
            for h in range(HEADS):
                a0 = h * CH
                a1 = a0 + CH
                if a1 <= 128:
                    nc.sync.dma_start(out=rkp[0:CH, h:h + 1],
                                      in_=rb[2][a0:a1, :])
                elif a0 >= 128:
                    nc.sync.dma_start(out=rkp[0:CH, h:h + 1],
                                      in_=rb[1][64 + a0 - 128:64 + a1 - 128, :])
                else:
                    m = 128 - a0
                    nc.sync.dma_start(out=rkp[0:m, h:h + 1],
                                      in_=rb[2][a0:128, :])
                    nc.sync.dma_start(out=rkp[m:CH, h:h + 1],
                                      in_=rb[1][64:64 + a1 - 128, :])
            nc.vector.tensor_mul(rkp[0:CH, :], rkp[0:CH, :], temp_sb[:])

            # stage GT into SBUF (DMA cannot read PSUM)
            gt_sb = [finpool.tile([128, C], F32, tag="gts0", name="gts0"),
                     finpool.tile([64, C], F32, tag="gts1", name="gts1")]
            nc.vector.tensor_copy(gt_sb[0][:], gt_ps[0][:])
            nc.vector.tensor_copy(gt_sb[1][:], gt_ps[1][:])
            # assemble A_T (32, HEADS*32): A_T[d, h*32+c] = GT[24h+d, 24h+c] * rk
            at = finpool.tile([32, HEADS * 32], F32, tag="at", name="at")
            nc.vector.memset(at[:], 0.0)
            for h in range(HEADS):
                a0 = h * CH
                a1 = a0 + CH
                col = slice(a0, a1)
                if a1 <= 128:
                    nc.sync.dma_start(out=at[0:CH, h * 32:h * 32 + CH],
                                      in_=gt_sb[0][a0:a1, col])
                elif a0 >= 128:
                    nc.sync.dma_start(out=at[0:CH, h * 32:h * 32 + CH],
                                      in_=gt_sb[1][a0 - 128:a1 - 128, col])
                else:
                    m = 128 - a0
                    nc.sync.dma_start(out=at[0:m, h * 32:h * 32 + CH],
                                      in_=gt_sb[0][a0:128, col])
                    nc.sync.dma_start(out=at[m:CH, h * 32:h * 32 + CH],
                                      in_=gt_sb[1][0:a1 - 128, col])
                nc.vector.tensor_scalar_mul(
                    at[0:CH, h * 32:h * 32 + CH],
                    at[0:CH, h * 32:h * 32 + CH],
                    rkp[0:CH, h:h + 1])

            # transpose per-head 32x32 blocks back: A[c, h*32+d]
            a_sb = finpool.tile([32, HEADS * 32], F32, tag="a", name="a")
            nc.vector.transpose(a_sb[:], at[:])
            e_sb = finpool.tile([32, HEADS * 32], F32, tag="e", name="e")
            nc.scalar.activation(e_sb[:], a_sb[:], AF.Exp)
            e3 = e_sb.rearrange("p (h d) -> p h d", d=32)
            sums = finpool.tile([CH, HEADS], F32, tag="sums", name="sums")
            nc.vector.tensor_reduce(
                sums[:], e3[0:CH, :, 0:CH], axis=mybir.AxisListType.X, op=AL.add)
            rs = finpool.tile([CH, HEADS], F32, tag="rs", name="rs")
            nc.vector.reciprocal(rs[:], sums[:])
            attn = finpool.tile([CH, HEADS * CH], BF16, tag="attn", name="attn")
            for h in range(HEADS):
                nc.vector.tensor_scalar_mul(
                    attn[:, h * CH:(h + 1) * CH],
                    e_sb[0:CH, h * 32:h * 32 + CH],
                    rs[:, h:h + 1])

            # dense blockdiag(attn) as two 96-row contraction blocks
            bd = [finpool.tile([96, C], BF16, tag="bd0", name="bd0"),
                  finpool.tile([96, C], BF16, tag="bd1", name="bd1")]
            nc.vector.memset(bd[0][:], 0.0)
            nc.vector.memset(bd[1][:], 0.0)
            for h in range(HEADS):
                nc.sync.dma_start(
                    out=bd[h // 4][(h % 4) * CH:(h % 4) * CH + CH,
                                   h * CH:(h + 1) * CH],
                    in_=attn[:, h * CH:(h + 1) * CH])
            # MT = blockdiag(attn)^T @ W_proj^T  (rows = v channels, 128+64)
            mt0_t = tpsum.tile([128, C], F32, tag="qt", name="mt0")
            mt1_t = tpsum.tile([128, C], F32, tag="kt", name="mt1")
            mt_ps = [mt0_t[:, :], mt1_t[0:64, :]]
            for mi, msl in enumerate((slice(0, 128), slice(128, 192))):
                for k in range(2):
                    nc.tensor.matmul(mt_ps[mi][:], bd[k][:, msl], wp_sb[k][:],
                                     start=(k == 0), stop=(k == 1))
            mt_sb = [finpool.tile([128, C], BF16, tag="mt_sb0", name="mt_sb0"),
                     finpool.tile([64, C], BF16, tag="mt_sb1", name="mt_sb1")]
            nc.vector.tensor_copy(mt_sb[0][:], mt_ps[0][:])
            nc.vector.tensor_copy(mt_sb[1][:], mt_ps[1][:])

            # ---- output: out = MT^T @ v (fused attn-apply + projection) ----
            for j in range(N // 512):
                col = slice(j * 512, (j + 1) * 512)
                for mi, (msz, msl) in enumerate(((128, slice(0, 128)),
                                                 (64, slice(128, 192)))):
                    ps = mmpsum.tile([msz, 512], F32, tag="mmps", name="mmps")
                    nc.tensor.matmul(ps[:], mt_sb[0][:, msl], v_sb[0][:, col],
                                     start=True, stop=False)
                    nc.tensor.matmul(ps[:], mt_sb[1][:, msl], v_sb[1][:, col],
                                     start=False, stop=True)
                    osb = outpool.tile([msz, 512], BF16, tag=f"osb{mi}",
                                       name=f"osb{mi}")
                    nc.scalar.activation(osb[:], ps[:], AF.Copy)
                    cs = 0 if mi == 0 else 128
                    nc.sync.dma_start(out=out_d[cs:cs + msz, col], in_=osb[:])
    nc.finalize()
    return nc


_NC_CACHE = {}


def _perm():
    # B0=q[0:128], B1=q[128:192]||k[128:192], B2=k[0:128], B3/B4=v
    return (list(range(0, 128)) + list(range(128, 192))
            + list(range(320, 384)) + list(range(192, 320))
            + list(range(384, 576)))


def kernel(x, feature, W_qkv, W_dw, W_proj, temperature):
    import ml_dtypes
    b = x.shape[0]
    perm = _perm()
    wq_p = np.asarray(W_qkv, np.float32)[perm, :]
    wq = np.ascontiguousarray(wq_p.T).astype(ml_dtypes.bfloat16)
    taps = np.ascontiguousarray(
        np.asarray(W_dw, np.float32).reshape(O, 9)[perm, :])
    vtaps = taps[384:576, :]  # v channels (original order)
    vd3 = np.zeros((128, 9 * 128), np.float32)
    for ti in range(9):
        vd3[:, ti * 128:(ti + 1) * 128][np.arange(128), np.arange(128)] = \
            vtaps[0:128, ti]
    vd4 = np.zeros((64, 9 * 64), np.float32)
    for ti in range(9):
        vd4[:, ti * 64:(ti + 1) * 64][np.arange(64), np.arange(64)] = \
            vtaps[128:192, ti]
    wp = np.ascontiguousarray(np.asarray(W_proj, np.float32).T).astype(
        ml_dtypes.bfloat16)
    temp = np.broadcast_to(
        np.asarray(temperature, np.float32).reshape(1, HEADS), (CH, HEADS))
    temp = np.ascontiguousarray(temp)

    if "nc" not in _NC_CACHE:
        _NC_CACHE["nc"] = build_nc()
    nc = _NC_CACHE["nc"]

    in_maps = []
    for i in range(b):
        in_maps.append({
            "x": np.ascontiguousarray(np.asarray(x[i], np.float32)),
            "f": np.ascontiguousarray(np.asarray(feature[i], np.float32)),
            "wq": wq, "taps": taps,
            "vdiag3": vd3.astype(ml_dtypes.bfloat16),
            "vdiag4": vd4.astype(ml_dtypes.bfloat16),
            "wp": wp, "temp": temp,
            "identb": np.eye(128, dtype=np.float32).astype(ml_dtypes.bfloat16),
            "identf": np.eye(128, dtype=np.float32),
        })
    res = run_bass_kernel_spmd(nc, in_maps, list(range(b)))
    outs = [np.asarray(r["out"], np.float32).reshape(C, H, W)
            for r in res.results]
    return np.stack(outs, axis=0)
